# revision 1
# baseline (speedup 1.0000x reference)
"""MultiHeadAttention kernel for Trainium2 (8 NeuronCores, data-parallel over batch).

Reference computation (B=8, S=2048, D=64, concat=768):
    q = x @ Wq.T ; k = x @ Wk.T ; v = x @ Wv.T          # [B,S,768]
    scores = (q @ k.T) / sqrt(64)                        # [B,S,S]  (full concat dim!)
    attn = softmax(scores, -1)
    out = (attn @ v) @ Wf.T + b                          # [B,S,64]

Key algebraic identity: since the scores contract over the FULL concat dim,
q @ k.T = x (Wq^T Wk) x^T with A := Wq^T Wk in R^{64x64}; similarly
(attn @ v) @ Wf^T = attn @ (x @ W2) with W2 := Wv^T Wf^T in R^{64x64}.
A and W2 are weight-only folds, precomputed on the host at load time (same
class as the bf16 cast / layout marshaling); all activation-dependent math
(y = A^T x^T, z = x W2 + b, scores, softmax, O') runs on device. The softmax
denominator comes from a ones column appended to z (bias folded into z, so
the final division yields attn@z + b directly).

Layout: scores are computed TRANSPOSED ([key chunk = 128 partitions, query
free]) so the exp output feeds the O' matmul directly. Score matmuls for two
key chunks are row-packed into PE row groups 0-63 / 64-127 (they execute
CONCURRENTLY on the array). x^T arrives host-transposed and duplicated onto
partitions 64-127; y^T is computed into both halves via tile_position
matmuls - no SBUF->SBUF duplication DMAs.

Scheduling: the O' matmul for pair p-1 is interleaved BETWEEN the score
matmuls of pair p, and the z matmuls run inside the prep loop, so the
score->exp pipeline (the critical sc-psum double-buffer rotation) never
waits behind unrelated PE work.

Precision: score matmuls are bf16; exp outputs and z are fp8e4 so the O'
accumulation runs in DoubleRow mode (2 key chunks per pass, 2x PE rate).
The exp is split across engines: ACT computes 20 of 32 [128 x 1024] blocks
exactly; 12 run on Vector as a Schraudolph fast exp (round(s*8*log2e + B)
written as uint8 IS the fp8e4 bit pattern of exp(s/8)).
PSUM accumulation and the normalize/store path stay fp32. Measured rel err
vs the fp32 reference ~7e-3 (gate 2e-2).
"""

import sys

sys.path.insert(0, "/opt/trn_rl_repo")

import ml_dtypes
import numpy as np

import concourse.bass as bass
import concourse.tile as tile
from concourse import bacc, mybir
from concourse.bass_utils import run_bass_kernel_spmd

F32 = mybir.dt.float32
F32R = mybir.dt.float32r
BF16 = mybir.dt.bfloat16
FP8 = mybir.dt.float8e4
U8 = mybir.dt.uint8
ALU = mybir.AluOpType
DRM = mybir.MatmulPerfMode.DoubleRow

B, S, D, C = 8, 2048, 64, 768
NCHUNK = S // 128          # 16 key chunks of 128
NPAIR = NCHUNK // 2        # 8 row-packed chunk pairs
NSUP = S // 512            # 4 query superblocks of 512
SCALING = 0.125            # 1/sqrt(64)
ZP = 80                    # z row pitch (DoubleRow needs step % 16 == 0)
# Schraudolph fast-exp constants (uint8 result IS the fp8e4 bit pattern);
# round-to-nearest convert measured on HW, c=0.35 tuned for it
SCH_A = float(SCALING * 8 * np.log2(np.e))
SCH_B = float(56.0 - 0.35)


def _vec_exp(p, jl):
    """Which exp blocks run on Vector (Schraudolph) instead of ACT."""
    return p >= 2 and jl == 1


def _build_nc():
    nc = bacc.Bacc("TRN2", target_bir_lowering=False, debug=False)

    a_d = nc.dram_tensor("a", [D, D], BF16, kind="ExternalInput")
    w2_d = nc.dram_tensor("w2", [128, D], BF16, kind="ExternalInput")
    xT_d = nc.dram_tensor("xT", [128, S], BF16, kind="ExternalInput")
    b_d = nc.dram_tensor("b_final", [D], F32, kind="ExternalInput")
    ident_d = nc.dram_tensor("ident", [128, 128], F32R, kind="ExternalInput")
    zeros_d = nc.dram_tensor("zeros", [1, 512], F32R, kind="ExternalInput")
    out_d = nc.dram_tensor("out", [S, D], F32, kind="ExternalOutput")

    with tile.TileContext(nc) as tc:
        _emit(tc, a_d, w2_d, xT_d, b_d, ident_d, zeros_d, out_d)
    nc.compile()
    return nc


def _emit(tc, a_d, w2_d, xT_d, b_d, ident_d, zeros_d, out_d):
    nc = tc.nc
    const = tc.alloc_tile_pool(name="const", bufs=1)

    # dep-free first PE instruction: triggers the PE IRAM instruction fetch
    # at t=0 instead of after the first operand DMA lands
    nc.tensor.nop(nofuse=True)

    # ---- sync (HWDGE) queue: a first (pe_warm + y gate), then x^T in
    # quarters so the j=0 chain starts as soon as cols 0-511 land
    a_sb = const.tile([D, D], BF16)
    nc.sync.dma_start(a_sb[:], a_d.ap())
    xTd = const.tile([128, S], BF16)
    xT_ap = xT_d.ap()
    for q in range(4):
        nc.sync.dma_start(xTd[:, q * 512 : (q + 1) * 512],
                          xT_ap[:, q * 512 : (q + 1) * 512])
    ident = const.tile([128, 128], F32R)
    nc.sync.dma_start(ident[:], ident_d.ap())

    # ---- gpsimd (SWDGE) queue: W2 (dup'd on host), bias, finalize consts
    w2_sb = const.tile([128, D], BF16)
    nc.gpsimd.dma_start(w2_sb[:], w2_d.ap())
    b_bcast = const.tile([128, D], F32)
    b_ap = b_d.ap()
    b_src = bass.AP(tensor=b_ap.tensor, offset=b_ap.offset, ap=[[0, 128]] + list(b_ap.ap))
    nc.gpsimd.dma_start(b_bcast[:], b_src)

    # z (DoubleRow stationary): cols 0-63 = x@W2+b, col 64 = ones (softmax
    # denominator), cols 65-79 = zero pad for the 16-aligned pitch
    z_sb = const.tile([128, NCHUNK, ZP], FP8)
    nc.gpsimd.memset(z_sb[:, :, D : D + 1], 1.0)
    nc.gpsimd.memset(z_sb[:, :, D + 1 : ZP], 0)

    # finalize ping-pong tiles, padding row 65 pre-zeroed (fp32r transposes
    # need an even innermost free count on the destination)
    ot_tiles = [const.tile([D + 2, 512], F32R, name=f"ot{i}") for i in range(2)]
    for i in range(2):
        nc.gpsimd.dma_start(ot_tiles[i][D + 1 : D + 2, :], zeros_d.ap())

    # warm the ACT exp table early so the table load overlaps the DMA phase
    warm = const.tile([1, 1], F32)
    nc.scalar.activation(out=warm[:], in_=a_sb[0:1, 0:2].bitcast(F32),
                         func=mybir.ActivationFunctionType.Exp, scale=1.0)

    yTd = const.tile([128, S], BF16)       # y^T = A^T x^T, rows 0-63 and 64-127

    # PSUM pool lifetimes chain (prep -> main -> finalize) without nesting:
    # prep_ps releases its banks to oacc; finalize transposes reuse sc slots.
    scp = tc.alloc_tile_pool(name="sc_ps", bufs=3, space="PSUM")
    etp = tc.alloc_tile_pool(name="et", bufs=3)
    pps = tc.alloc_tile_pool(name="prep_ps", bufs=1, space="PSUM")

    pe_warm = pps.tile([128, 512], F32, tag="t2", bufs=2, name="pe_warm")
    nc.tensor.matmul(pe_warm[0:32, 0:32], a_sb[0:32, 0:32], a_sb[0:32, 0:32],
                     start=True, stop=True)

    def scores_exp_j(p, jg, jl, eT):
        # jg = global query superblock (0-3), jl = slot within the half (0/1)
        n0, n1 = 2 * p, 2 * p + 1
        sc = scp.tile([128, 1024], F32, tag="sc", name=f"sc{p}_{jg}")
        nc.tensor.matmul(sc[:, 0:512], xTd[0:D, n0 * 128 : (n0 + 1) * 128],
                         yTd[0:D, jg * 512 : (jg + 1) * 512],
                         start=True, stop=True)
        nc.tensor.matmul(sc[:, 512:1024], xTd[D:128, n1 * 128 : (n1 + 1) * 128],
                         yTd[D:128, jg * 512 : (jg + 1) * 512],
                         start=True, stop=True)
        if _vec_exp(p, jl):
            # Schraudolph fast exp on Vector: round(s*A + B) as uint8 bits
            nc.vector.tensor_scalar(eT[:, jl, :, :].bitcast(U8), sc[:],
                                    SCH_A, SCH_B, ALU.mult, ALU.add)
        else:
            nc.scalar.activation(out=eT[:, jl, :, :], in_=sc[:],
                                 func=mybir.ActivationFunctionType.Exp,
                                 scale=SCALING)

    def new_eT(h, p):
        # [keys 128][half-local j 2][chunk 2][512]
        return etp.tile([128, 2, 2, 512], FP8, tag="et", bufs=3, name=f"eT{h}_{p}")

    def z_pair(h):
        # z chunks 2h (xT rows 0-63) / 2h+1 (rows 64-127); bias folded on the
        # psum->sbuf copy (fp8 out). Uses prep-pool scratch so the z matmuls
        # can run during prep without touching the O' accumulator banks.
        n0, n1 = 2 * h, 2 * h + 1
        zp0 = pps.tile([128, 512], F32, tag="t2", bufs=2, name=f"zp{n0}")
        zp1 = pps.tile([128, 512], F32, tag="t2", bufs=2, name=f"zp{n1}")
        nc.tensor.matmul(zp0[:, 0:D], xTd[0:D, n0 * 128 : (n0 + 1) * 128],
                         w2_sb[0:D, :], start=True, stop=True)
        nc.tensor.matmul(zp1[:, 0:D], xTd[D:128, n1 * 128 : (n1 + 1) * 128],
                         w2_sb[D:128, :], start=True, stop=True)
        nc.vector.tensor_add(z_sb[:, n0, 0:D], zp0[:, 0:D], b_bcast[:])
        nc.vector.tensor_add(z_sb[:, n1, 0:D], zp1[:, 0:D], b_bcast[:])

    # prep: per j-block, the two y^T matmuls (both partition halves via
    # tile_position), their copies, the half-0 pair-0 scores, and the z pairs
    eT00 = new_eT(0, 0)
    for j in range(NSUP):
        yp = pps.tile([128, 512], F32, tag="t2", bufs=2, name=f"yp{j}")
        nc.tensor.matmul(yp[0:D, :], a_sb[:], xTd[0:D, j * 512 : (j + 1) * 512],
                         start=True, stop=True)
        nc.tensor.matmul(yp[D:128, :], a_sb[:], xTd[0:D, j * 512 : (j + 1) * 512],
                         start=True, stop=True)
        nc.vector.tensor_copy(yTd[0:D, j * 512 : (j + 1) * 512], yp[0:D, :])
        nc.scalar.copy(yTd[D:128, j * 512 : (j + 1) * 512], yp[D:128, :])
        if j < 2:
            scores_exp_j(0, j, j, eT00)
    for h4 in range(4):
        z_pair(h4)

    pps.release()

    # ---- main loop over two query halves (o_ps = 2 banks -> sc pool gets
    # 3 buffers, which hides the score->exp psum rotation latency). The O'
    # matmul for the previous (half, pair) slot rides between the score
    # matmuls of the current slot; the previous half's last O' + finalize
    # interleave with the next half's first scores.
    oacc_pool = tc.alloc_tile_pool(name="oacc", bufs=1, space="PSUM")
    fsb = tc.alloc_tile_pool(name="fin_sb", bufs=2)
    osb = tc.alloc_tile_pool(name="out_sb", bufs=2)
    out_r = out_d.ap().rearrange("(j q p) d -> j p q d", p=128, q=4)

    def oprime_j(p, jl, eT, o_ps):
        nc.tensor.matmul(o_ps[jl][:], z_sb[:, 2 * p : 2 * p + 2, :],
                         eT[:, jl, :, :],
                         start=(p == 0), stop=(p == NPAIR - 1),
                         perf_mode=DRM)

    def finalize_j(h, jl, o_ps):
        jg = 2 * h + jl
        ot = ot_tiles[jl]
        nc.vector.tensor_copy(ot[0 : D + 1, 0:256], o_ps[jl][0 : D + 1, 0:256])
        nc.scalar.copy(ot[0 : D + 1, 256:512], o_ps[jl][0 : D + 1, 256:512])
        # reuses the (now idle) score-psum slots for the transposes
        pt = scp.tile([128, 4, D + 2], F32R, tag="sc", name=f"fin{jg}")
        for q in range(4):
            nc.tensor.transpose(pt[:, q, :], ot[:, q * 128 : (q + 1) * 128],
                                ident[0 : D + 2, 0 : D + 2])
        r_sb = fsb.tile([128, 4], F32, tag="r")
        nc.vector.reciprocal(r_sb[:], pt[:, :, D : D + 1].bitcast(F32))
        o_out = osb.tile([128, 4, D], F32, tag="oo")
        nc.vector.tensor_mul(o_out[:], pt[:, :, 0:D],
                             r_sb[:].unsqueeze(2).broadcast_to([128, 4, D]))
        nc.sync.dma_start(out_r[jg], o_out[:])

    prev = None   # (p, eT, o_ps) of the previous slot within the half
    fin_prev = None  # (h, o_ps) of the previous half awaiting last O'+finalize
    for h in range(2):
        o_ps = [oacc_pool.tile([ZP, 512], F32, tag=f"o{jl}", name=f"o_ps{h}_{jl}")
                for jl in range(2)]
        for p in range(NPAIR):
            if h == 0 and p == 0:
                prev = (0, eT00, o_ps)
                continue
            eT = new_eT(h, p)
            for jl in range(2):
                scores_exp_j(p, 2 * h + jl, jl, eT)
                if prev is not None:
                    pp, peT, po = prev
                    oprime_j(pp, jl, peT, po)
                elif fin_prev is not None:
                    # previous half's last pair O' + its finalize
                    ph, po = fin_prev
                    oprime_j(NPAIR - 1, jl, prev_last_eT, po)
                    finalize_j(ph, jl, po)
            if prev is None:
                fin_prev = None
            prev = (p, eT, o_ps)
            if h == 0 and 1 <= p <= 4:
                # deferred z pairs 4-7, two bank-aligned outputs per sc slot
                zh = p + 3
                n0, n1 = 2 * zh, 2 * zh + 1
                zp = scp.tile([128, 1024], F32, tag="sc", name=f"zpd{zh}")
                nc.tensor.matmul(zp[:, 0:D], xTd[0:D, n0 * 128 : (n0 + 1) * 128],
                                 w2_sb[0:D, :], start=True, stop=True)
                nc.tensor.matmul(zp[:, 512 : 512 + D],
                                 xTd[D:128, n1 * 128 : (n1 + 1) * 128],
                                 w2_sb[D:128, :], start=True, stop=True)
                nc.vector.tensor_add(z_sb[:, n0, 0:D], zp[:, 0:D], b_bcast[:])
                nc.vector.tensor_add(z_sb[:, n1, 0:D], zp[:, 512 : 512 + D],
                                     b_bcast[:])
        # end of half: hand the last pair's O' to the next half's first slot
        pp, prev_last_eT, po = prev
        fin_prev = (h, po)
        prev = None

    # tail: last half's final O' + finalize
    ph, po = fin_prev
    for jl in range(2):
        oprime_j(NPAIR - 1, jl, prev_last_eT, po)
        finalize_j(ph, jl, po)

    osb.release()
    fsb.release()
    oacc_pool.release()
    etp.release()
    scp.release()
    const.release()


_NC_CACHE = {}


def _get_nc():
    if "nc" not in _NC_CACHE:
        _NC_CACHE["nc"] = _build_nc()
    return _NC_CACHE["nc"]


def kernel(x, w_q, w_k, w_v, w_final, b_final, _trace=False):
    nc = _get_nc()
    bfr = lambda a: np.asarray(a, dtype=np.float32).astype(ml_dtypes.bfloat16).astype(np.float32)
    # weight-only folds (load-time preprocessing): A = Wq^T Wk, W2 = Wv^T Wf^T
    A = (bfr(w_q).T @ bfr(w_k)).astype(ml_dtypes.bfloat16)
    W2 = (bfr(w_v).T @ bfr(w_final).T).astype(ml_dtypes.bfloat16)
    W2d = np.ascontiguousarray(np.concatenate([W2, W2], axis=0))  # both halves
    xb = np.asarray(x, dtype=np.float32).astype(ml_dtypes.bfloat16)  # [B,S,D]
    # host-side layout marshaling: x^T per batch, duplicated onto both
    # partition halves for the row-packed score matmuls
    xT = np.ascontiguousarray(np.concatenate([xb.transpose(0, 2, 1),
                                              xb.transpose(0, 2, 1)], axis=1))
    shared = {
        "a": np.ascontiguousarray(A),
        "w2": W2d,
        "b_final": np.ascontiguousarray(np.asarray(b_final, dtype=np.float32)),
        "ident": np.eye(128, dtype=np.float32),
        "zeros": np.zeros((1, 512), dtype=np.float32),
    }
    in_maps = [dict(shared, xT=xT[b]) for b in range(B)]
    res = run_bass_kernel_spmd(nc, in_maps, core_ids=list(range(B)), trace=_trace)
    out = np.stack([res.results[b]["out"] for b in range(B)], axis=0)
    if _trace:
        return out, res
    return out

